# revision 39
# baseline (speedup 1.0000x reference)
"""Multi-head attention (N=2, S=4096, E=768, H=8 heads, D=96) + output projection,
sharded over 8 Trainium2 NeuronCores.

Sharding: data-parallel over query rows. Core i handles batch n = i//4 and query
rows (i%4)*1024 .. +1024 of that batch, attending over the batch's full K/V.
No collectives needed; the host concatenates the 8 output chunks.

Device algorithm per core (all matmuls bf16 on TensorE, f32 PSUM accumulation):
  sT[k,q]   = K_h @ Q_h^T          (scores, transposed layout: k on partitions)
  pT        = exp(sT * scale)       (ScalarE, PSUM->SBUF, bf16 out)
  ctxT_aug  = V_aug^T-contract pT   (V augmented with a ones column -> row 96 of
                                     the output is the softmax denominator)
  ctxn      = ctxT * (1/denom)      (DVE recip + GpSimd partition_broadcast)
  Y         = sum_h ctxn_h^T @ W_h^T + b   (fc_out, row-accumulated in PSUM)

Schedule: ScalarE (exp, ~15.6us/head) is the pacer; the PE runs ~13.7us/head of
score+ctx matmuls, leaving ~1.9us/head of PE slack. The fc_out matmuls (20.5us
total) are drip-fed into that slack via two-stage per-unit partials: each of the
8 (row-tile, half) units per chunk accumulates heads 0..n-1 into an SBUF partial
as soon as those heads' ctxn are normalized, so the post-exp tail is only ~11
matmuls + bias adds + final DMA.

Host pre-arranges layouts and pre-casts to bf16 (halves HBM traffic and keeps
all loads on the hardware DGE queues; GpSimd then only runs partition
broadcasts, so no Q7 library switches mid-kernel).
"""

import numpy as np
import sys

for _p in ("/opt/trn_rl_repo",):
    if _p not in sys.path:
        sys.path.append(_p)

import ml_dtypes
import concourse.bass as bass
import concourse.tile as tile
from concourse import bacc, mybir
from concourse.bass_utils import run_bass_kernel_spmd

F32 = mybir.dt.float32
BF16 = mybir.dt.bfloat16

N_CORES = 8
NB = 2          # batch
S = 4096        # key/value sequence length
SQ = 1024       # query rows per core
E = 768
H = 8
D = 96
KT = S // 128   # 32 k-tiles of 128
SCALE = float(np.float32(1.0) / np.sqrt(np.float32(D)))  # matches reference

# exp staging: k-tile group sizes (PSUM banks per sT tile); sum must be KT.
# Small first group -> the first exp fires sooner at each head start.
EXP_GROUPS = [2] + [3] * 10
assert sum(EXP_GROUPS) == KT

# fc_out interleave schedule. Units are (qt, half): row qt*128, cols half*384.
# Each entry: (qc, h) -> list of (chunk, unit, nh_target) emitted in that
# head's PE slack (at gi slots 3/6/9). A unit at nh_target accumulates heads
# nh_old..nh_target-1 onto its running SBUF partial (bias folded in at nh=0).
# nh_target == 8 finalizes into y_sb (DMA'd once both halves of a qt land).
# Constraint: an emission at (qc1, h) may only read ctxn heads < h (that
# head's norm lands during the next head's first flushes). Budget: <=12
# extra 384-col matmuls per head (~1.9us PE slack vs ScalarE pacing).
FC_SLOTS = {
    (0, 4): [(0, 0, 4), (0, 1, 4)],
    (0, 5): [(0, 2, 4), (0, 3, 4)],
    (0, 6): [(0, 4, 5), (0, 5, 5)],
    (0, 7): [(0, 6, 5), (0, 7, 5)],
    (1, 0): [(0, 0, 8), (0, 1, 8)],
    (1, 1): [(0, 2, 8), (0, 3, 8)],
    (1, 2): [(0, 4, 8), (0, 5, 8), (1, 0, 2)],
    (1, 3): [(0, 6, 8), (0, 7, 8), (1, 1, 3)],
    (1, 4): [(1, 2, 4), (1, 3, 4), (1, 0, 4)],
    (1, 5): [(1, 4, 5), (1, 5, 5), (1, 1, 5)],
    (1, 6): [(1, 6, 6), (1, 0, 6), (1, 2, 6)],
    (1, 7): [(1, 7, 7), (1, 1, 7), (1, 3, 7)],
}
# gi positions (within a head's 11 exp groups) where fc emissions fire. The
# first slot sits at gi=4: head h-1's norm (recip -> GpSimd broadcast -> mul,
# ~2.5us after its last ctx matmul) must complete before an fc unit reading
# ctxn[h-1] enters the in-order PE queue, or the whole score stream stalls
# behind it. Slot spacing >=2 groups (~2.6us of score/ctx matmuls) hides each
# unit's PSUM-slot round trip (matmul drain -> DVE add -> WAR release ~1.3us).
SLOT_GIS = (4, 7, 9)
# chunk-0 units fully finalize mid-stream; chunk-1 units stop at nh<=7 and get
# their last head + finalization in the tail (pre-tail advance + quartered norm)
_nh = {(c, u): 0 for c in range(2) for u in range(8)}
for slots in FC_SLOTS.values():
    for c, u, nh in slots:
        assert nh > _nh[(c, u)]
        _nh[(c, u)] = nh
assert all(nh == H for (c, u), nh in _nh.items() if c == 0)
assert all(nh <= H - 1 for (c, u), nh in _nh.items() if c == 1)


def build_nc():
    nc = bacc.Bacc("TRN2", target_bir_lowering=False, debug=False)

    kT_d = nc.dram_tensor("kT", [H, D, S], BF16, kind="ExternalInput")
    qT_d = nc.dram_tensor("qT", [H, D, SQ], BF16, kind="ExternalInput")
    va_d = nc.dram_tensor("va", [H, 128, KT, D + 1], BF16, kind="ExternalInput")
    wt_d = nc.dram_tensor("wt", [E, E], BF16, kind="ExternalInput")  # fc_w.T
    bias_d = nc.dram_tensor("bias", [1, E], F32, kind="ExternalInput")
    y_d = nc.dram_tensor("y", [SQ, E], F32, kind="ExternalOutput")

    with tile.TileContext(nc) as tc:
        with (
            tc.tile_pool(name="persist", bufs=1) as persist,
            tc.tile_pool(name="pt", bufs=3) as pt_pool,
            tc.tile_pool(name="norm", bufs=2) as norm_pool,
            tc.tile_pool(name="normq", bufs=4) as nq_pool,
            tc.tile_pool(name="yout", bufs=2) as y_pool,
            tc.tile_pool(name="yhalf", bufs=4) as yh_pool,
            tc.tile_pool(name="ypart", bufs=8) as ypart_pool,
            tc.tile_pool(name="psbig", bufs=2, space="PSUM") as ps_big,
            tc.tile_pool(name="pssm", bufs=2, space="PSUM") as ps_sm,
        ):
            # ---- persistent SBUF tensors ----
            kT = persist.tile([D, H, S], BF16, tag="kT")          # 64 KB/part
            qT = persist.tile([D, H, SQ], BF16, tag="qT")         # 16 KB/part
            va = persist.tile([128, H, KT, D + 1], BF16, tag="va")  # 48.5 KB/part
            wt_sb = persist.tile([D, H, E], BF16, tag="wt")       # 12 KB/part
            ctxn = persist.tile([D, H, SQ], BF16, tag="ctxn")     # 16 KB/part
            bias_sb = persist.tile([1, E], F32, tag="bias1")
            bias_b = persist.tile([128, E], F32, tag="bias")      # 3 KB/part

            # ---- loads (HWDGE; sync + scalar queues) ----
            # All queues are gated by the framework's ~7us startup barriers,
            # and a single queue serializes trigger+transfer, so the critical
            # head-0 chunks are split across BOTH HWDGE queues (sync and
            # scalar — the scalar queue is idle until the first exp anyway).
            nc.scalar.dma_start(out=qT[:, 0, 0:512], in_=qT_d[0, :, 0:512])
            nc.sync.dma_start(out=kT[:, 0, 0:512], in_=kT_d[0, :, 0:512])
            nc.sync.dma_start(out=kT[:, 0, 512:2048], in_=kT_d[0, :, 512:2048])
            nc.sync.dma_start(out=qT[:, 0, 512:1024], in_=qT_d[0, :, 512:1024])
            nc.sync.dma_start(out=va[:, 0, 0:16, :], in_=va_d[0, :, 0:16, :])
            nc.sync.dma_start(out=kT[:, 0, 2048:], in_=kT_d[0, :, 2048:])
            nc.sync.dma_start(out=va[:, 0, 16:, :], in_=va_d[0, :, 16:, :])
            nc.sync.dma_start(out=bias_sb, in_=bias_d[0:1, :])
            # warms the Q7 broadcast library long before the first norm needs it
            nc.gpsimd.partition_broadcast(bias_b, bias_sb)
            for h in range(1, H):
                nc.sync.dma_start(out=kT[:, h, 0:2048], in_=kT_d[h, :, 0:2048])
                nc.sync.dma_start(out=qT[:, h, :], in_=qT_d[h])
                nc.sync.dma_start(out=va[:, h, 0:16, :], in_=va_d[h, :, 0:16, :])
                nc.sync.dma_start(out=kT[:, h, 2048:], in_=kT_d[h, :, 2048:])
                nc.sync.dma_start(out=va[:, h, 16:, :], in_=va_d[h, :, 16:, :])
                if h == 2:
                    for hh in range(H):
                        nc.sync.dma_start(
                            out=wt_sb[:, hh, :], in_=wt_d[hh * D:(hh + 1) * D, :]
                        )

            # ---- fc_out unit machinery ----
            # y_part[(chunk, u)] = (sbuf partial tile, heads accumulated)
            y_part = {}
            # y_row[(chunk, qt)] = [y_sb tile, halves remaining]
            y_row = {}

            def emit_fc_unit(chunk, u, nh_new, pool=None):
                qt, half = divmod(u, 2)
                row = chunk * 512 + qt * 128
                hs = half * 384
                prev = y_part.pop((chunk, u), None)
                nh_old = prev[1] if prev is not None else 0
                y_pp = (pool or ps_sm).tile([128, 384], F32, tag="sm" if pool is None else "sT", name="y_pp")
                for h in range(nh_old, nh_new):
                    nc.tensor.matmul(
                        y_pp,
                        ctxn[:, h, row:row + 128],
                        wt_sb[:, h, hs:hs + 384],
                        start=(h == nh_old), stop=(h == nh_new - 1),
                    )
                addend = prev[0] if prev is not None else bias_b[:, hs:hs + 384]
                if nh_new == H:
                    if chunk == 1:
                        # tail: alternate the adds between DVE and GpSimd (both
                        # otherwise idle) and the half-row DMAs between the two
                        # HWDGE queues, so the last transfer lands ~3us earlier
                        y_hb = yh_pool.tile([128, 384], F32, tag="yh", name="y_hb")
                        nc.vector.tensor_add(y_hb, y_pp, addend)
                        dq = nc.scalar if u % 2 else nc.sync
                        dq.dma_start(
                            out=y_d[row:row + 128, hs:hs + 384], in_=y_hb
                        )
                        return
                    ent = y_row.get((chunk, qt))
                    if ent is None:
                        ent = [y_pool.tile([128, E], F32, tag="y", name="y_sb"), 2]
                        y_row[(chunk, qt)] = ent
                    y_sb = ent[0]
                    nc.vector.tensor_add(y_sb[:, hs:hs + 384], y_pp, addend)
                    ent[1] -= 1
                    if ent[1] == 0:
                        nc.sync.dma_start(out=y_d[row:row + 128, :], in_=y_sb)
                        del y_row[(chunk, qt)]
                elif prev is not None:
                    # in-place update: keeps one ypart tile per unit alive for
                    # its whole lifetime (a fresh tile per stage can deadlock —
                    # the add that would free the old slot is the same add
                    # waiting for a new slot)
                    nc.vector.tensor_add(prev[0], y_pp, prev[0])
                    y_part[(chunk, u)] = (prev[0], nh_new)
                else:
                    yp = ypart_pool.tile([128, 384], F32, tag="ypart")
                    nc.vector.tensor_add(yp, y_pp, addend)
                    y_part[(chunk, u)] = (yp, nh_new)

            # ---- softmax normalization ----
            def emit_norm(ctx_ps, h, qs):
                # row D of ctx_ps is the denominator (ones column of va).
                # recip_approx is a bitwise custom-DVE op: PSUM reads corrupt
                # it, so bounce the row through SBUF first.
                recip = norm_pool.tile([1, 512], F32, tag="recip")
                nc.vector.tensor_copy(recip, ctx_ps[D:D + 1, :])
                nc.vector.reciprocal_approx_fast(recip, recip)
                bcast = norm_pool.tile([D, 512], F32, tag="bcast")
                nc.gpsimd.partition_broadcast(bcast, recip)
                nc.vector.tensor_mul(
                    ctxn[:, h, qs:qs + 512], ctx_ps[0:D, :], bcast
                )

            def emit_norm_last_recip(ctx_ps, qq):
                # one 128-column quarter's reciprocal + partition broadcast
                c0 = qq * 128
                rq = nq_pool.tile([1, 128], F32, tag="recq", name="rq")
                nc.vector.tensor_copy(rq, ctx_ps[D:D + 1, c0:c0 + 128])
                nc.vector.reciprocal_approx_fast(rq, rq)
                bq = nq_pool.tile([D, 128], F32, tag="bcq", name="bq")
                nc.gpsimd.partition_broadcast(bq, rq)
                return bq

            def emit_norm_last_unit_pair(ctx_ps, h, qs, qq, bq):
                # normalize quarter qq and finalize its two fc units. Tail
                # y_pp tiles come from the (now idle) score PSUM pool so
                # consecutive units' PSUM round trips overlap.
                c0 = qq * 128
                nc.vector.tensor_mul(
                    ctxn[:, h, qs + c0:qs + c0 + 128],
                    ctx_ps[0:D, c0:c0 + 128], bq,
                )
                emit_fc_unit(1, 2 * qq, H, pool=ps_big)
                emit_fc_unit(1, 2 * qq + 1, H, pool=ps_big)

            # ---- main attention loop ----
            # Software-pipelined ctx matmuls: lag the exp stream by 2 groups so
            # the in-order PE queue never waits on an exp at block boundaries.
            pend = []   # (ctx_ps, h, qs, kt0, pt, is_last_group)

            def flush_one():
                c_ps, c_h, c_qs, c_kt0, c_pt, c_last = pend.pop(0)
                g = c_pt.shape[1] // 512
                for j in range(g):
                    kt = c_kt0 + j
                    nc.tensor.matmul(
                        c_ps,
                        va[:, c_h, kt, :],
                        c_pt[:, j * 512:(j + 1) * 512],
                        start=(kt == 0), stop=(kt == KT - 1),
                    )
                if c_last:
                    emit_norm(c_ps, c_h, c_qs)

            for qc in range(2):
                qs = qc * 512
                for h in range(H):
                    ctx_ps = ps_sm.tile([D + 1, 512], F32, tag="sm")
                    slots = FC_SLOTS.get((qc, h), [])
                    kt0 = 0
                    for gi, g in enumerate(EXP_GROUPS):
                        sT = ps_big.tile([128, g * 512], F32, tag="sT")
                        for j in range(g):
                            kt = kt0 + j
                            nc.tensor.matmul(
                                sT[:, j * 512:(j + 1) * 512],
                                kT[:, h, kt * 128:(kt + 1) * 128],
                                qT[:, h, qs:qs + 512],
                                start=True, stop=True,
                            )
                        pt = pt_pool.tile([128, g * 512], BF16, tag="pt")
                        nc.scalar.activation(
                            pt, sT, mybir.ActivationFunctionType.Exp, scale=SCALE
                        )
                        pend.append(
                            (ctx_ps, h, qs, kt0, pt, gi == len(EXP_GROUPS) - 1)
                        )
                        while len(pend) > 2:
                            flush_one()
                        kt0 += g
                        if gi in SLOT_GIS:
                            si = SLOT_GIS.index(gi)
                            if si < len(slots):
                                emit_fc_unit(*slots[si])
            # ---- tail ----
            # pend holds the last head's final two groups. Flush g9, then use
            # the window while exp(g10) finishes to advance the first
            # quarters' units to nh=7 (reads heads <=6 only), flush g10's ctx,
            # and run the quartered final norm software-pipelined one quarter
            # deep: quarter q's mul + unit finalizations are emitted right
            # after quarter q+1's recip, so the DVE never idles on a GpSimd
            # broadcast and the first fc finals start ~1.5us after the last
            # ctx matmul.
            flush_one()
            c_ps, c_h, c_qs, c_kt0, c_pt, c_last = pend.pop(0)
            assert c_last and not pend
            pre_tail = [(c, u) for (c, u), nh in _nh.items() if nh < H - 1]
            for c, u in pre_tail[:2]:
                emit_fc_unit(c, u, H - 1, pool=ps_big)
            g = c_pt.shape[1] // 512
            for j in range(g):
                kt = c_kt0 + j
                nc.tensor.matmul(
                    c_ps,
                    va[:, c_h, kt, :],
                    c_pt[:, j * 512:(j + 1) * 512],
                    start=(kt == 0), stop=(kt == KT - 1),
                )
            bq0 = emit_norm_last_recip(c_ps, 0)
            bq1 = emit_norm_last_recip(c_ps, 1)
            emit_norm_last_unit_pair(c_ps, c_h, c_qs, 0, bq0)
            for c, u in pre_tail[2:]:
                emit_fc_unit(c, u, H - 1, pool=ps_big)
            bq2 = emit_norm_last_recip(c_ps, 2)
            emit_norm_last_unit_pair(c_ps, c_h, c_qs, 1, bq1)
            bq3 = emit_norm_last_recip(c_ps, 3)
            emit_norm_last_unit_pair(c_ps, c_h, c_qs, 2, bq2)
            emit_norm_last_unit_pair(c_ps, c_h, c_qs, 3, bq3)
            assert not y_part and not y_row

    nc.finalize()
    return nc


def _prep_inputs(values, keys, query, fc_w, fc_b):
    """Build per-core input maps (host-side sharding + layout + bf16 cast)."""
    values = np.ascontiguousarray(values, dtype=np.float32)
    keys = np.ascontiguousarray(keys, dtype=np.float32)
    query = np.ascontiguousarray(query, dtype=np.float32)
    wt = np.ascontiguousarray(
        np.asarray(fc_w, dtype=np.float32).T.astype(ml_dtypes.bfloat16)
    )
    bias = np.ascontiguousarray(np.asarray(fc_b, dtype=np.float32).reshape(1, E))

    per_batch = []
    for n in range(NB):
        # K -> [H, D, S]
        kTn = np.ascontiguousarray(
            keys[n].reshape(S, H, D).transpose(1, 2, 0).astype(ml_dtypes.bfloat16)
        )
        # V -> [H, 128, KT, D+1] with ones in the last column
        # (partition-contiguous: per head, each of the 128 partitions reads
        #  KT*(D+1) contiguous elements -> large DMA descriptors)
        van = np.empty((H, 128, KT, D + 1), dtype=ml_dtypes.bfloat16)
        van[..., :D] = values[n].reshape(KT, 128, H, D).transpose(2, 1, 0, 3).astype(
            ml_dtypes.bfloat16
        )
        van[..., D] = 1.0
        per_batch.append((kTn, van))

    in_maps = []
    for core in range(N_CORES):
        n = core // (N_CORES // NB)
        qi = core % (N_CORES // NB)
        qrows = query[n, qi * SQ:(qi + 1) * SQ]
        qTn = np.ascontiguousarray(
            qrows.reshape(SQ, H, D).transpose(1, 2, 0).astype(ml_dtypes.bfloat16)
        )
        kTn, van = per_batch[n]
        in_maps.append({
            "kT": kTn, "qT": qTn, "va": van, "wt": wt, "bias": bias,
        })
    return in_maps


def _assemble(results):
    y = np.empty((NB, S, E), dtype=np.float32)
    for core in range(N_CORES):
        n = core // (N_CORES // NB)
        qi = core % (N_CORES // NB)
        y[n, qi * SQ:(qi + 1) * SQ] = results[core]["y"]
    return y


def run(values, keys, query, fc_w, fc_b, **spmd_kwargs):
    nc = build_nc()
    in_maps = _prep_inputs(values, keys, query, fc_w, fc_b)
    res = run_bass_kernel_spmd(nc, in_maps, core_ids=list(range(N_CORES)),
                               **spmd_kwargs)
    return _assemble(res.results), res


def kernel(values, keys, query, fc_w, fc_b):
    y, _ = run(values, keys, query, fc_w, fc_b)
    return y
